# revision 2
# baseline (speedup 1.0000x reference)
"""Trainium2 Bass kernel for nn_BruteForceUpdater (associativity + fp16).

Reference computation:
    xs = x[:, 0, :]                       # [256, 128]
    U  = (xs @ W1.T) @ W2.T               # [256, 8256]
    fw_{i+1} = sigmoid(10*(fw_i + U_i - 0.5))   (serial over batch)
    pred_i = fw2_i @ relu(fw1_i @ x_i)    # fw1 = fw[:8192].reshape(64,128)

Key algebraic restructuring vs v1: U = xs @ (W2 @ W1).T. Each core owns
1032 fast-weight rows R = [1024c, 1024(c+1)) u [8192+8c, 8192+8c+8):
    Weq_R.T = sum_k W1[kblk].T-as-stationary @ W2[R, kblk].T   [128, 1032]
    U_R.T   = Weq_R.T-chunks-as-stationary @ xs.T              9x [128, 256]
Phase W streams W2slice.T ++ W1 blocks as one fp16 tensor over BOTH HW
DGE queues (sync + act engines, alternating k parity). Phase U lands U.T
directly in the scan layout (no transposes). The 256-step sigmoid scan
and the prediction tail then run as in v1, with the tail's departition
done by one SBUF->SBUF DMA instead of a DRAM round-trip.
"""
import os
import sys

sys.path.insert(0, "/opt/trn_rl_repo")

import numpy as np
from contextlib import ExitStack

import concourse.bass as bass
import concourse.tile as tile
from concourse import mybir
from concourse.bass_utils import run_bass_kernel_spmd

F32 = mybir.dt.float32
F32R = mybir.dt.float32r
F16 = mybir.dt.float16
AF = mybir.ActivationFunctionType

IN = 128
HID = 64
NFW = IN * HID + HID          # 8256
B = 256
K2 = 2 * NFW                  # 16512
KT = K2 // 128                # 129 contraction tiles
NCORES = 8
MT_OWN = 8                    # full 128-row fw tiles owned per core
NT = MT_OWN + 1               # + 8-row shared-tail tile
MSL = MT_OWN * 128            # 1024 streamed W2T cols (own rows only;
                              # the 8 shared rows' Weq comes via consts)
WC = MSL + 128                # streamed tile width (W2T cols + W1 block)
GAIN, SHIFT = 10.0, 0.5

NSLOT = 16                    # stream ring slots
LOOK = 12                     # DMA lookahead (<= NSLOT - 2)
PCH = 64                      # pred-product column chunk width

_NC_CACHE = None


def _build_bass():
    nc = bass.Bass("TRN2", target_bir_lowering=False, debug=False)

    wc_d = nc.dram_tensor("wcomb", [K2, WC], F16, kind="ExternalInput")
    c16_d = nc.dram_tensor("c16", [128, B + 8], F16, kind="ExternalInput")
    c32_d = nc.dram_tensor("c32", [128, B + NT + 1], F32, kind="ExternalInput")
    pred_d = nc.dram_tensor("pred", [1, B], F32, kind="ExternalOutput")
    dbg = bool(int(os.environ.get("KDBG", "0")))
    if dbg:
        u_dbg_d = nc.dram_tensor("u_dbg", [128, NT * B], F32,
                                 kind="ExternalOutput")
        fw_dbg_d = nc.dram_tensor("fw_dbg", [128, NT * B], F32,
                                  kind="ExternalOutput")
        weq_dbg_d = nc.dram_tensor("weq_dbg", [128, MSL], F16,
                                   kind="ExternalOutput")

    with tile.TileContext(nc) as tc:
        with ExitStack() as ctx:
            const_pool = ctx.enter_context(tc.tile_pool(name="const", bufs=1))
            stream_pool = ctx.enter_context(tc.tile_pool(name="wcs", bufs=1))
            big_pool = ctx.enter_context(tc.tile_pool(name="big", bufs=1))

            c16 = const_pool.tile([128, B + 8], F16)
            xst16 = c16[:, 0:B]
            weqsh_t = c16[:, B:B + 8]           # host-computed Weq_sh.T
            c32 = const_pool.tile([128, B + NT + 1], F32)
            xst = c32[:, 0:B]                       # f32 copy for pred muls
            fw0_t = c32[:, B:B + NT]
            ones_t = c32[:, B + NT:B + NT + 1]
            ones16 = const_pool.tile([128, 1], F16)
            bias_t = const_pool.tile([128, 1], F32)

            wbuf = stream_pool.tile([128, NSLOT * WC], F16)    # stream ring
            weq_sb = big_pool.tile([128, MSL], F16)
            u_sb = big_pool.tile([128, NT * B], F32)
            fw_sb = big_pool.tile([128, NT * B], F32)
            t_big = big_pool.tile([128, 2 * NT], F32)
            prod_big = big_pool.tile([128, MT_OWN * B], F16)
            h_flat = big_pool.tile([1, MT_OWN * B], F32)
            h_sb = big_pool.tile([MT_OWN, B], F32)
            r_sb = big_pool.tile([MT_OWN, B], F32)
            p_sb = big_pool.tile([MT_OWN, B], F16)
            pred_sb = big_pool.tile([1, B], F32)

            u_r = u_sb[:].rearrange("p (m i) -> p m i", m=NT)
            # fw is STEP-major: col = i*NT + m, so the scan's reads and
            # writes of one step are 9 contiguous elements
            fw_r = fw_sb[:].rearrange("p (i m) -> p m i", m=NT)

            def wslot(j):
                s = j % NSLOT
                return wbuf[:, s * WC:(s + 1) * WC]

            # phase-U psum tiles: 9 x 1KB at 1KB steps; two tiles share each
            # 2KB bank -> bank-first gets start=True, bank-last stop=True.
            u_first = [m % 2 == 0 for m in range(NT)]
            u_last = [m % 2 == 1 or m == NT - 1 for m in range(NT)]

            csem = nc.alloc_semaphore("csem")
            dsem = [nc.alloc_semaphore(f"dsem{s}") for s in range(NSLOT)]
            pe_sem = nc.alloc_semaphore("pe")
            sv = nc.alloc_semaphore("sv")     # DVE progress
            sa = nc.alloc_semaphore("sa")     # ACT progress
            pp = nc.alloc_semaphore("pp")     # PE pred/phase progress
            dsm = nc.alloc_semaphore("dsm")   # tail-phase DMA

            with tc.tile_pool(name="pw", bufs=1, space="PSUM") as pw_pool, \
                 tc.tile_pool(name="pu", bufs=1, space="PSUM") as pu_pool:
                # psum_weq: banks 0-1 (one 1024-wide accumulation region)
                psum_weq = pw_pool.tile([128, 2 * 512], F32)
                # psum_u: banks 3-7 (9 x 256-f32 tiles at 1KB steps)
                psum_u = pu_pool.tile([128, NT * B], F32)
                # tail psums alias regions that are dead by the tail phase
                psum_h = psum_u[0:1, 0:MT_OWN * B]
                psum_p = psum_weq[0:1, 0:B]

                with tc.tile_critical():
                    svc = [0]                 # sv value tracker

                    def dve_inc(inst):
                        inst.then_inc(sv, 1)
                        svc[0] += 1
                        return svc[0]

                    # constants via gpsimd (SWDGE queue, parallel to streams)
                    nc.gpsimd.dma_start(c16[:], c16_d[:, :]).then_inc(csem, 16)
                    nc.gpsimd.dma_start(c32[:], c32_d[:, :]).then_inc(csem, 16)
                    nc.vector.memset(bias_t[:], -GAIN * SHIFT)
                    c1 = nc.vector.tensor_copy(ones16[:], ones_t)
                    c1._wait_ge(csem, 32)
                    dve_inc(c1)

                    def dma_k(j):
                        eng = nc.sync if (j % 2 == 0) else nc.scalar
                        d = eng.dma_start(
                            wslot(j), wc_d[j * 128:(j + 1) * 128, :])
                        if j >= NSLOT:
                            # slot free once phase-W(j-NSLOT) fully read it
                            d._wait_ge(pe_sem, j - NSLOT + 1)
                        d.then_inc(dsem[j % NSLOT], 16)

                    for j in range(LOOK):
                        dma_k(j)

                    # ---- phase W: Weq_R.T accumulation over 129 k-tiles ----
                    # two 512-wide chunk matmuls per k-tile (ISA caps the
                    # moving free dim at 512)
                    for k in range(KT):
                        if k + LOOK < KT:
                            dma_k(k + LOOK)
                        for ci in range(2):
                            mm = nc.tensor.matmul(
                                psum_weq[:, ci * 512:(ci + 1) * 512],
                                wslot(k)[:, MSL:WC],
                                wslot(k)[:, ci * 512:(ci + 1) * 512],
                                start=(k == 0), stop=(k == KT - 1),
                            )
                            if ci == 0:
                                mm._wait_ge(dsem[k % NSLOT],
                                            16 * (k // NSLOT + 1))
                            else:
                                mm.then_inc(pe_sem, 1)

                    # ---- phase U: U_R.T = WeqT-chunks @ xs.T ----
                    # c1 above waited csem>=32, so any later sv value
                    # transitively implies the consts are loaded
                    cpw = nc.vector.tensor_copy(
                        weq_sb[:, 0:1024], psum_weq[:, 0:1024])
                    cpw._wait_ge(pe_sem, KT)
                    v_w2 = dve_inc(cpw)
                    for m in range(NT):
                        lhs = (weq_sb[:, m * 128:(m + 1) * 128]
                               if m < MT_OWN else weqsh_t)
                        um = nc.tensor.matmul(
                            psum_u[0:(128 if m < MT_OWN else 8),
                                   m * B:(m + 1) * B],
                            lhs, xst16[:],
                            start=u_first[m], stop=u_last[m],
                        )
                        if m == 0:
                            um._wait_ge(sv, v_w2)
                        if m == NT - 1:
                            um.then_inc(pp, 1)

                    # zero the shared-tail tile (only lanes 0:8 hold data),
                    # then copy the full tiles and the 8 tail lanes from psum
                    msz = nc.vector.memset(u_sb[:, (NT - 1) * B:NT * B], 0.0)
                    dve_inc(msz)
                    cpu = nc.vector.tensor_copy(
                        u_sb[:, 0:MT_OWN * B], psum_u[:, 0:MT_OWN * B])
                    cpu._wait_ge(pp, 1)
                    dve_inc(cpu)
                    cpt = nc.vector.tensor_copy(
                        u_sb[0:8, (NT - 1) * B:NT * B],
                        psum_u[0:8, (NT - 1) * B:NT * B])
                    v_ucp = dve_inc(cpt)

                    # ---- 256-step sigmoid scan, with the prediction
                    # products (DVE) and column sums (PE) interleaved into
                    # the scan's idle engine windows ----
                    def prod_mul(m, c, wait_sa=None):
                        pr = nc.vector.tensor_mul(
                            prod_big[:, m * B + c * PCH:m * B + (c + 1) * PCH],
                            fw_r[:, m, c * PCH:(c + 1) * PCH],
                            xst[:, c * PCH:(c + 1) * PCH])
                        if wait_sa is not None:
                            pr._wait_ge(sa, wait_sa)
                        v = dve_inc(pr)
                        hm = nc.tensor.matmul(
                            psum_h[0:1, m * B + c * PCH:m * B + (c + 1) * PCH],
                            ones16[:],
                            prod_big[:, m * B + c * PCH:m * B + (c + 1) * PCH],
                            start=(c == 0 and m % 2 == 0),
                            stop=(c == 3 and m % 2 == 1),
                        )
                        hm._wait_ge(sv, v)
                        if c == 3 and m == MT_OWN - 1:
                            hm.then_inc(pp, 1)

                    for i in range(B):
                        t_t = t_big[:, (i % 2) * NT:(i % 2) * NT + NT]
                        prev = fw0_t if i == 0 else fw_r[:, :, i - 1]
                        add = nc.vector.tensor_add(t_t, prev, u_r[:, :, i])
                        if i > 0:
                            add._wait_ge(sa, i)
                        else:
                            add._wait_ge(sv, v_ucp)
                        v_add = dve_inc(add)
                        act = nc.scalar.activation(
                            fw_r[:, :, i], t_t, AF.Sigmoid,
                            bias=bias_t[:], scale=GAIN)
                        act._wait_ge(sv, v_add)
                        act.then_inc(sa, 1)
                        # slot a prediction-product chunk into the idle
                        # window while the ACT engine runs this sigmoid:
                        # chunk (m, c) needs fw cols <= 64c+63, final once
                        # i >= 64c+65
                        for c in range(3):
                            if 64 * c + 66 <= i < 64 * c + 66 + MT_OWN:
                                prod_mul(i - (64 * c + 66), c)

                    # ---- prediction tail (last column chunk + finish) ----
                    for m in range(MT_OWN):
                        prod_mul(m, 3, wait_sa=B if m == 0 else None)
                    cph = nc.vector.tensor_copy(h_flat[:], psum_h[:])
                    cph._wait_ge(pp, 2)
                    v_hflat = dve_inc(cph)
                    # departition [1, 8*256] -> [8, 256] via SBUF->SBUF DMA
                    dh = nc.sync.dma_start(h_sb[:], h_flat[0:1, :])
                    dh._wait_ge(sv, v_hflat)
                    dh.then_inc(dsm, 16)
                    rl = nc.vector.tensor_relu(r_sb[:], h_sb[:])
                    rl._wait_ge(dsm, 16)
                    v_relu = dve_inc(rl)
                    pm = nc.vector.tensor_mul(p_sb[:], r_sb[:],
                                              fw_r[0:MT_OWN, NT - 1, :])
                    pm._wait_ge(sv, v_relu)
                    v_psb = dve_inc(pm)
                    pmm = nc.tensor.matmul(
                        psum_p[:], ones16[0:MT_OWN, :],
                        p_sb[:], start=True, stop=True)
                    pmm._wait_ge(sv, v_psb)
                    pmm.then_inc(pp, 1)
                    cp3 = nc.vector.tensor_copy(pred_sb[:], psum_p[:])
                    cp3._wait_ge(pp, 3)
                    v_pred = dve_inc(cp3)
                    dout = nc.sync.dma_start(pred_d[:, :], pred_sb[:])
                    dout._wait_ge(sv, v_pred)
                    dout.then_inc(dsm, 16)
                    if dbg:
                        du = nc.sync.dma_start(u_dbg_d[:, :], u_sb[:])
                        du._wait_ge(sv, v_pred)
                        du.then_inc(dsm, 16)
                        df = nc.sync.dma_start(fw_dbg_d[:, :], fw_sb[:])
                        df._wait_ge(sv, v_pred)
                        df.then_inc(dsm, 16)
                        dw = nc.sync.dma_start(weq_dbg_d[:, :], weq_sb[:])
                        dw._wait_ge(sv, v_pred)
                        dw.then_inc(dsm, 16)

    _dedupe_waits(nc)
    return nc


def _dedupe_waits(nc):
    """Collapse duplicate semaphore waits the framework occasionally emits
    (walrus allows very few sync commands per instruction)."""
    for fnn in nc.m.functions:
        for blk in fnn.blocks:
            for inst in blk.instructions:
                si = inst.sync_info
                if si is None or not si.on_wait or len(si.on_wait) < 2:
                    continue
                best = {}
                order = []
                for w in si.on_wait:
                    if w.wait_reg is not None or w.wait_mode != "sem-ge-imm":
                        key = ("raw", id(w))
                    else:
                        key = (w.sync_type, w.id, w.wait_mode)
                    if key not in best:
                        best[key] = w
                        order.append(key)
                    elif (w.wait_value or 0) > (best[key].wait_value or 0):
                        best[key] = w
                deduped = [best[k] for k in order]
                if len(deduped) != len(si.on_wait):
                    inst.sync_info = mybir.SyncInfo(
                        on_wait=deduped, on_update=si.on_update)


def _split_noops(nc):
    """Split multi-wait NoOps into single-wait chains (walrus's CTRL_NO
    struct carries very few sync commands)."""
    if getattr(nc, "_noops_split", False):
        return
    nc._noops_split = True
    split_id = [0]
    for fnn in nc.m.functions:
        for blk in fnn.blocks:
            out = []
            changed = False
            for inst in blk.instructions:
                si = inst.sync_info
                if (type(inst).__name__ == "InstNoOp" and si is not None
                        and len(si.on_wait) > 1):
                    changed = True
                    for w in si.on_wait[:-1]:
                        no = mybir.InstNoOp(
                            name=f"noop_waitsplit_{split_id[0]}",
                            text_hint="waitsplit")
                        split_id[0] += 1
                        no.engine = inst.engine
                        no.sync_info = mybir.SyncInfo(
                            on_wait=[w], on_update=[])
                        out.append(no)
                    inst.sync_info = mybir.SyncInfo(
                        on_wait=[si.on_wait[-1]], on_update=si.on_update)
                out.append(inst)
            if changed:
                blk.instructions = out


def _get_nc():
    global _NC_CACHE
    if _NC_CACHE is None:
        _NC_CACHE = _build_bass()
    return _NC_CACHE


def _make_in_maps(x, W1, W2, fw0):
    xs = np.ascontiguousarray(x[:, 0, :].astype(np.float32))       # [256, 128]
    xst = np.ascontiguousarray(xs.T)                                # [128, 256]
    xst16 = xst.astype(np.float16)
    W1 = np.asarray(W1, dtype=np.float32)
    W2 = np.asarray(W2, dtype=np.float32)
    fw0 = np.asarray(fw0, dtype=np.float32)
    W1_16 = W1.astype(np.float16)                                   # [16512, 128]

    W1_f = W1_16.astype(np.float32)
    in_maps = []
    for c in range(NCORES):
        own16 = W2[c * 1024:(c + 1) * 1024, :].astype(np.float16)   # [1024, 16512]
        wcomb = np.concatenate(
            [np.ascontiguousarray(own16.T), W1_16], axis=1)         # [16512, 1152]
        # the 8 shared fast-w2 rows' Weq contribution, computed host-side
        # with the same fp16-inputs/fp32-accumulate numerics as the device
        sh16 = W2[MT_OWN * 128 * NCORES + 8 * c:
                  MT_OWN * 128 * NCORES + 8 * c + 8, :].astype(np.float16)
        weq_sh = sh16.astype(np.float32) @ W1_f                     # [8, 128]
        c16 = np.zeros((128, B + 8), np.float16)
        c16[:, 0:B] = xst16
        c16[:, B:B + 8] = weq_sh.T.astype(np.float16)
        fw0_t = np.zeros((128, NT), np.float32)
        for m in range(MT_OWN):
            fw0_t[:, m] = fw0[c * 1024 + m * 128: c * 1024 + (m + 1) * 128]
        fw0_t[0:8, NT - 1] = fw0[MT_OWN * 128 * NCORES + 8 * c:
                                 MT_OWN * 128 * NCORES + 8 * c + 8]
        c32 = np.zeros((128, B + NT + 1), np.float32)
        c32[:, 0:B] = xst
        c32[:, B:B + NT] = fw0_t
        c32[:, B + NT] = 1.0
        in_maps.append({
            "wcomb": np.ascontiguousarray(wcomb),
            "c16": c16,
            "c32": c32,
        })
    return in_maps


def kernel(x, W1, W2, fw0, _trace=False, _tmpdir=None):
    nc = _get_nc()
    _split_noops(nc)
    in_maps = _make_in_maps(x, W1, W2, fw0)
    res = run_bass_kernel_spmd(
        nc, in_maps, core_ids=list(range(NCORES)),
        trace=_trace, tmpdir=_tmpdir,
    )
    preds = np.zeros((1, B), np.float64)
    for c in range(NCORES):
        preds += res.results[c]["pred"].astype(np.float64)
    out = preds.astype(np.float32).reshape(B, 1)
    if _trace:
        return out, res
    return out
